# revision 35
# baseline (speedup 1.0000x reference)
"""Trainium2 Bass kernel for nn_Attention_40037685133427.

FiLM-conditioned LayerNorm + 16-head self-attention (B=2, N=2048, D=1024),
tensor-parallel over 8 NeuronCores: core c owns heads {2c, 2c+1}.

v2 redesign, driven by microbenchmarks on this silicon (back-to-back 512-col
bf16 matmuls issue every ~216ns; 64-contraction quadrant pairs co-execute;
engine access patterns must start at 32-aligned partitions):
  - LN stats via DVE bn_stats/bn_aggr on a token-major copy of x (no PE
    matmuls, no serial chains, no DRAM round trips); batched Newton rsqrt.
  - per-token (u=rstd, m=mean*rstd, sd=1/u) row triplets are PE-transposed
    once per slice; u broadcasts to a [128,512] tile via a selector matmul,
    and (m, sd) ride the QKV accumulation as PE outer-products against a
    [3,128] (0, -sum(W_g), sum(beta'W)) stationary, so the whole LN+FiLM
    correction costs one DVE multiply per projection.
  - QKV runs on raw x as 3 bank-interleaved accumulation chains.
  - attention: S^T = K Q^T head-pairs co-execute in PE quadrants; one fused
    exp per (jt, slice) on ACT (scale folded, no max subtraction); attn@V
    uses 65-col stationaries [V | ones] so both heads' softmax denominators
    ride along; evacuation uses partition-shifted DVE copies, fast-approx
    reciprocals, and a zero-padded [33,128] selector matmul broadcasts 1/den.
  - normalize + output projection are emitted one slice late so their PE work
    never blocks the (ACT-exp-bound) attention stream; the steady-state
    attention phase runs with zero exp stalls (~142us for 128 exp tiles).
Host sums the 8 partial y^T outputs (row-split Wo => partial sums).
FiLM weights load as 8 wide DMAs on the sync queue feeding 4 parallel
accumulation chains; the scalar DMA queue is reserved for the token-major
x tiles so bn_stats starts immediately.
Measured: 320.1us HW exec (traced) per core (prep ~156us PE-bound, attention
~143us at the ACT-exp floor with zero stalls), rel L2 error 0.0054 vs fp32
reference; baseline v1 was 614us traced / 540.7us untraced.
"""

import sys

sys.path.insert(0, "/opt/trn_rl_repo")

import numpy as np
import ml_dtypes

import concourse.bass as bass
from concourse import bacc
import concourse.tile as tile
from concourse import mybir
from concourse.bass_utils import run_bass_kernel_spmd
from concourse.masks import make_identity

f32 = mybir.dt.float32
bf16 = mybir.dt.bfloat16
fp16 = mybir.dt.float16
AF = mybir.ActivationFunctionType
ALU = mybir.AluOpType

B, N, DIM = 2, 2048, 1024
HEADS, DH = 16, 64
TOK = B * N            # 4096 tokens, batch-major
KT = DIM // 128        # 8 k-tiles over the model dim
NSL = TOK // 512       # 8 token slices of 512
JT = N // 128          # 16 key tiles per batch
COND = 1024
NCORES = 8


def build_program():
    nc = bacc.Bacc("TRN2", target_bir_lowering=False, debug=False)

    xT = nc.dram_tensor("xT", [DIM, TOK], bf16, kind="ExternalInput").ap()
    xN = nc.dram_tensor("xN", [TOK, DIM], bf16, kind="ExternalInput").ap()
    ceT = nc.dram_tensor("ceT", [128, 2 * KT], f32, kind="ExternalInput").ap()
    gammaT = nc.dram_tensor("gammaT", [128, KT], f32, kind="ExternalInput").ap()
    condW = nc.dram_tensor("condW", [COND, 2 * DIM], bf16, kind="ExternalInput").ap()
    condb = nc.dram_tensor("condb", [2, 2 * DIM], f32, kind="ExternalInput").ap()
    wqkv = nc.dram_tensor("wqkv", [DIM, 384], bf16, kind="ExternalInput").ap()
    wo = nc.dram_tensor("wo", [128, DIM], bf16, kind="ExternalInput").ap()

    yT_out = nc.dram_tensor("yT", [DIM, TOK], bf16, kind="ExternalOutput").ap()

    with tile.TileContext(nc) as tc:
        with (
            tc.tile_pool(name="const", bufs=1) as const,
            tc.tile_pool(name="persist", bufs=1) as persist,
            tc.tile_pool(name="big", bufs=1) as bigp,
            tc.tile_pool(name="work", bufs=3) as work,
            tc.tile_pool(name="ps", bufs=8, space="PSUM") as ps,
        ):
            # PSUM tags: st2 2x[128,1024] (4 banks), po 2x[128,512] (2 banks),
            # ps 2x[128,512] (2 banks) = 8 banks
            def ps_st2(shape=(128, 1024), dtype=f32):
                return ps.tile(list(shape), dtype, tag="st2", bufs=2, name="st2t")

            def ps_po(shape=(128, 512), dtype=f32):
                return ps.tile(list(shape), dtype, tag="po", bufs=2, name="pot")

            def ps_sm(shape=(128, 512), dtype=f32):
                return ps.tile(list(shape), dtype, tag="ps", bufs=2, name="pst")

            # ---------------- constants / weights ----------------
            ident = const.tile([128, 128], bf16)
            make_identity(nc, ident[:])
            ident32 = const.tile([128, 128], f32)
            make_identity(nc, ident32[:])
            ones_col = const.tile([128, 1], bf16)
            nc.vector.memset(ones_col[:], 1.0)
            ones_col_h = const.tile([128, 1], fp16)
            nc.vector.memset(ones_col_h[:], 1.0)
            ones1 = const.tile([1, 128], bf16)
            nc.vector.memset(ones1[:], 1.0)
            # [33,128] selector: row 0 -> partitions 0:64 (head0), row 32 ->
            # 64:128 (head1); rows 1..31 are zero so junk moving rows vanish.
            ones2v = const.tile([33, 128], bf16)
            nc.vector.memset(ones2v[:], 0.0)
            nc.vector.memset(ones2v[0:1, 0:64], 1.0)
            nc.vector.memset(ones2v[32:33, 64:128], 1.0)

            wo_bf = persist.tile([128, DIM], bf16, tag="wo")
            nc.sync.dma_start(wo_bf[:], wo)
            wg_raw = []
            for kt in range(KT):
                wg = persist.tile([128, 384], bf16, tag="wg", bufs=KT, name=f"wg{kt}")
                nc.sync.dma_start(wg[:], wqkv[kt * 128:(kt + 1) * 128, :])
                wg_raw.append(wg)
            # x loaded as 16 [128, 2048] tiles: (kt, batch-half)
            x16 = [[None, None] for _ in range(KT)]
            for kt in range(KT):
                for bh in range(2):
                    xr = bigp.tile([128, 2048], bf16, tag="x16", bufs=16,
                                   name=f"x{kt}_{bh}")
                    nc.sync.dma_start(xr[:], xT[kt * 128:(kt + 1) * 128,
                                            bh * 2048:(bh + 1) * 2048])
                    x16[kt][bh] = xr

            def xsl(kt, isl):
                bh, q = isl // 4, isl % 4
                return x16[kt][bh][:, q * 512:(q + 1) * 512]

            gam = const.tile([128, KT], f32)
            nc.gpsimd.dma_start(gam[:], gammaT)
            cet = const.tile([128, 2 * KT], f32)
            nc.gpsimd.dma_start(cet[:], ceT)

            # ---------------- FiLM conditioning ----------------
            sil = const.tile([128, 2 * KT], f32)
            # silu(x) = x / (1 + exp(-x)) via Exp (single ACT table set)
            nc.scalar.activation(sil[:], cet[:], AF.Exp, scale=-1.0)
            nc.vector.tensor_scalar(sil[:], sil[:], 1.0, None, ALU.add)
            silr = const.tile([128, 2 * KT], f32)
            nc.vector.reciprocal_approx_fast(silr[:], sil[:])
            nc.vector.tensor_tensor(sil[:], silr[:], cet[:], op=ALU.mult)
            sil_bf = const.tile([128, 2 * KT], bf16)
            nc.vector.tensor_copy(sil_bf[:], sil[:])

            gp = const.tile([128, 2 * KT], f32)   # scale^T, col = b*KT + kt
            bp = const.tile([128, 2 * KT], f32)   # shift^T
            # 4 film chains accumulate in parallel over kt so one wide condW
            # DMA per kt (on the otherwise-idle sync queue) feeds all of them
            pcs = [ps_po((2, 512)), ps_po((2, 512)), ps_sm((2, 512)), ps_sm((2, 512))]
            for kt in range(KT):
                cwk = work.tile([128, 2048], bf16, tag="cwk", bufs=2)
                nc.sync.dma_start(cwk[:], condW[kt * 128:(kt + 1) * 128, :])
                for cs in range(4):
                    nc.tensor.matmul(pcs[cs][:], sil_bf[:, 2 * kt:2 * kt + 2],
                                     cwk[:, cs * 512:(cs + 1) * 512],
                                     start=(kt == 0), stop=(kt == KT - 1))
            csls = []
            for cs in range(4):
                cbw = work.tile([2, 512], f32, tag="cbw", bufs=4)
                nc.gpsimd.dma_start(cbw[:], condb[:, cs * 512:(cs + 1) * 512])
                csl = work.tile([2, 512], f32, tag="csl", bufs=4)
                nc.vector.tensor_tensor(csl[:], pcs[cs][:], cbw[:], op=ALU.add)
                csls.append(csl)
            for cs in range(4):
                dst = gp if cs < 2 else bp
                for c in range(4):
                    tp = ps_sm((128, 2))
                    nc.tensor.matmul(tp[:], csls[cs][0:2, c * 128:(c + 1) * 128],
                                     ident32[0:2, 0:2], is_transpose=True,
                                     start=True, stop=True)
                    k = (4 * cs + c) % KT
                    nc.vector.tensor_copy(dst[:, k::KT], tp[:])
            gpf = const.tile([128, 2 * KT], f32)
            nc.vector.tensor_scalar(gpf[:], gp[:], 1.0, None, ALU.add)
            for b in range(B):
                sl = slice(b * KT, (b + 1) * KT)
                nc.vector.tensor_tensor(gpf[:, sl], gpf[:, sl], gam[:], op=ALU.mult)
            bpb = const.tile([128, 2 * KT], bf16)
            nc.vector.tensor_copy(bpb[:], bp[:])

            # ---------------- per-batch folded QKV weights + column sums ----------------
            wscaled = [[None] * KT for _ in range(B)]
            wgs_neg, wbs = [], []
            for b in range(B):
                pgs = ps_po((1, 512))
                pbs = ps_po((1, 512))
                for kt in range(KT):
                    col = b * KT + kt
                    nc.tensor.matmul(pbs[0:1, 0:384], bpb[:, col:col + 1], wg_raw[kt][:],
                                     start=(kt == 0), stop=(kt == KT - 1))
                    wsb = persist.tile([128, 384], bf16, tag="wsb", bufs=B * KT,
                                       name=f"wsb{b}_{kt}")
                    nc.vector.tensor_scalar(wsb[:], wg_raw[kt][:], gpf[:, col:col + 1],
                                            None, ALU.mult)
                    nc.tensor.matmul(pgs[0:1, 0:384], ones_col[:], wsb[:],
                                     start=(kt == 0), stop=(kt == KT - 1))
                    wscaled[b][kt] = wsb
                gsr = work.tile([1, 512], f32, tag="wrow", bufs=2)
                nc.vector.tensor_copy(gsr[0:1, 0:384], pgs[0:1, 0:384])
                bsr = work.tile([1, 512], f32, tag="wrow", bufs=2)
                nc.vector.tensor_copy(bsr[0:1, 0:384], pbs[0:1, 0:384])
                wtp = ps_sm((128, 3))
                for c in range(3):
                    nc.tensor.matmul(wtp[:, c:c + 1], gsr[0:1, c * 128:(c + 1) * 128],
                                     ident32[0:1, 0:1], is_transpose=True,
                                     start=True, stop=True)
                wg_n = const.tile([128, 3], f32, name=f"wgn{b}")
                nc.vector.tensor_scalar(wg_n[:], wtp[:], -1.0, None, ALU.mult)
                wgs_neg.append(wg_n)
                wtp2 = ps_sm((128, 3))
                for c in range(3):
                    nc.tensor.matmul(wtp2[:, c:c + 1], bsr[0:1, c * 128:(c + 1) * 128],
                                     ident32[0:1, 0:1], is_transpose=True,
                                     start=True, stop=True)
                wb_c = const.tile([128, 3], f32, name=f"wbc{b}")
                nc.vector.tensor_copy(wb_c[:], wtp2[:])
                wbs.append(wb_c)

            # [3,128] per-(batch,proj) correction stationaries: rows
            # (0, wgs_neg, wbs); contracted with the (u, m, 1) row triplet
            # they add m*wgs_neg + wbs into the QKV accumulation on the PE.
            wqksel = [[None] * 3 for _ in range(B)]
            for b in range(B):
                for p in range(3):
                    wqc = work.tile([128, 3], bf16, tag="wqc", bufs=2)
                    nc.vector.memset(wqc[:, 0:1], 0.0)
                    nc.vector.tensor_copy(wqc[:, 1:2], wgs_neg[b][:, p:p + 1])
                    nc.vector.tensor_copy(wqc[:, 2:3], wbs[b][:, p:p + 1])
                    pwq = ps_sm((3, 128), bf16)
                    nc.tensor.matmul(pwq[:], wqc[:], ident[:],
                                     is_transpose=True, start=True, stop=True)
                    wsel = const.tile([3, 128], bf16, name=f"wqksel{b}_{p}")
                    nc.vector.tensor_copy(wsel[:], pwq[:])
                    wqksel[b][p] = wsel

            # ---------------- LN stats via DVE bn_stats on token-major x ----------------
            # bn_stats/bn_aggr produce per-token (mean, var) directly; no PE
            # matmuls, no serial accumulation chains, no transposes needed.
            stagMV = [const.tile([128, 32], f32, name=f"stagmv{b}") for b in range(B)]

            def stats_chunk(b, q4):
                for q in range(4 * q4, 4 * q4 + 4):   # 4 token-tiles of 128
                    tt = b * 16 + q
                    xn = work.tile([128, DIM], bf16, tag="xn", bufs=6)
                    nc.scalar.dma_start(xn[:], xN[tt * 128:(tt + 1) * 128, :])
                    bnb = work.tile([128, 12], f32, tag="bnb", bufs=3)
                    nc.vector.bn_stats(bnb[:, 0:6], xn[:, 0:512])
                    nc.vector.bn_stats(bnb[:, 6:12], xn[:, 512:1024])
                    nc.vector.bn_aggr(stagMV[b][:, 2 * q:2 * q + 2], bnb[:])

            # ---------------- stage 2: batched Newton rsqrt + broadcast back ----------------
            # per-slice (u, m, 1) row triplets: u broadcasts to U_sb via a
            # selector matmul; (m, 1) rows ride the QKV accumulation as
            # outer-product corrections (see wqksel below).
            U_sb, umr_sb = [None] * NSL, [None] * NSL
            selc = const.tile([128, 3], bf16)
            nc.vector.memset(selc[:, 0:1], 1.0)
            nc.vector.memset(selc[:, 1:3], 0.0)
            sel_u = const.tile([3, 128], bf16)
            psel = ps_sm((3, 128), bf16)
            nc.tensor.matmul(psel[:], selc[:], ident[:],
                             is_transpose=True, start=True, stop=True)
            nc.vector.tensor_copy(sel_u[:], psel[:])

            def stage2_b(b):
                mean = work.tile([128, 16], f32, tag="mean", bufs=2)
                nc.vector.tensor_copy(mean[:], stagMV[b][:, 0::2])
                var = work.tile([128, 16], f32, tag="var", bufs=2)
                nc.vector.tensor_scalar(var[:], stagMV[b][:, 1::2], 1.0, 1e-5, ALU.mult, ALU.add)
                y = work.tile([128, 16], f32, tag="yt", bufs=2)
                nc.vector.tensor_scalar(y[:], var[:], -0.5, 1.5, ALU.mult, ALU.add)
                t = work.tile([128, 16], f32, tag="tt", bufs=2)
                for _ in range(2):
                    nc.vector.tensor_tensor(t[:], y[:], y[:], op=ALU.mult)
                    nc.vector.tensor_tensor(t[:], t[:], var[:], op=ALU.mult)
                    nc.vector.tensor_scalar(t[:], t[:], -0.5, 1.5, ALU.mult, ALU.add)
                    nc.vector.tensor_tensor(y[:], y[:], t[:], op=ALU.mult)
                m = work.tile([128, 16], f32, tag="mt", bufs=2)
                nc.vector.tensor_tensor(m[:], mean[:], y[:], op=ALU.mult)
                sd = work.tile([128, 16], f32, tag="sd", bufs=2)
                nc.vector.tensor_tensor(sd[:], var[:], y[:], op=ALU.mult)
                stg2 = const.tile([128, 48], bf16, name=f"stg2_{b}")
                nc.vector.tensor_copy(stg2[:, 0::3], y[:])
                nc.vector.tensor_copy(stg2[:, 1::3], m[:])
                nc.vector.tensor_copy(stg2[:, 2::3], sd[:])
                for q in range(4):
                    isl = b * 4 + q
                    umps = ps_sm((3, 512), bf16)
                    for c in range(4):
                        tt = q * 4 + c
                        nc.tensor.matmul(umps[0:3, c * 128:(c + 1) * 128],
                                         stg2[:, 3 * tt:3 * tt + 3], ident[:],
                                         is_transpose=True, start=True, stop=True)
                    umr = persist.tile([3, 512], bf16, tag="umr", bufs=NSL,
                                       name=f"umr{isl}")
                    nc.vector.tensor_copy(umr[:], umps[:])
                    umr_sb[isl] = umr
                    pu = ps_sm()
                    nc.tensor.matmul(pu[:], sel_u[:], umr[:], start=True, stop=True)
                    ub = persist.tile([128, 512], bf16, tag="Usb", bufs=NSL)
                    nc.vector.tensor_copy(ub[:], pu[:])
                    U_sb[isl] = ub

            # ---------------- QKV on raw x (LN+FiLM affine folded into weights) ----------------
            # per-batch tensors so batch-1 writes never alias batch-0 reads
            q2Tb = [persist.tile([128, N], bf16, tag="q2T", bufs=B, name=f"q2T{b}")
                    for b in range(B)]
            k2Tb = [persist.tile([128, N], bf16, tag="k2T", bufs=B, name=f"k2T{b}")
                    for b in range(B)]
            V2 = [None] * (B * JT)

            def qkv_isl(isl):
                b, q = isl // 4, isl % 4
                sl = slice(q * 512, (q + 1) * 512)
                # interleaved accumulation chains on 3 PSUM banks; st2 slots
                # are free before attention, widening the rotation so chain
                # k+1 never waits on chain k's evacuation
                pqs = {2: ps_st2((128, 512)), 1: ps_st2((128, 512)), 0: ps_po()}
                for kt in range(KT):
                    for p in (2, 1, 0):
                        nc.tensor.matmul(pqs[p][:], wscaled[b][kt][:, p * 128:(p + 1) * 128],
                                         xsl(kt, isl),
                                         start=(kt == 0), stop=False)
                for p in (2, 1, 0):
                    nc.tensor.matmul(pqs[p][:], wqksel[b][p][:], umr_sb[isl][:],
                                     start=False, stop=True)
                # evacuate all three chains first so the PSUM slots free up
                vtile = work.tile([128, 512], bf16, tag="vtile", bufs=3)
                nc.vector.tensor_tensor(vtile[:], pqs[2][:], U_sb[isl][:], op=ALU.mult)
                nc.vector.tensor_tensor(k2Tb[b][:, sl], pqs[1][:], U_sb[isl][:], op=ALU.mult)
                nc.vector.tensor_tensor(q2Tb[b][:, sl], pqs[0][:], U_sb[isl][:], op=ALU.mult)
                for q4 in range(4):
                    jt = isl * 4 + q4
                    pv = ps_sm((128, 128), bf16)
                    nc.tensor.matmul(pv[:], vtile[:, q4 * 128:(q4 + 1) * 128],
                                     ident[:], is_transpose=True,
                                     start=True, stop=True)
                    va = persist.tile([128, 65], bf16, tag="Va0", bufs=B * JT)
                    nc.vector.tensor_copy(va[:, 0:64], pv[:, 0:64])
                    nc.vector.memset(va[:, 64:65], 1.0)
                    vh = persist.tile([128, 65], bf16, tag="Vh1", bufs=B * JT)
                    nc.vector.tensor_copy(vh[:, 0:64], pv[:, 64:128])
                    nc.vector.memset(vh[:, 64:65], 1.0)
                    V2[jt] = (va, vh)

            # prep: batch-0 chain, then batch-1; bn of b1 overlaps b0 QKV
            stats_chunk(0, 0)
            stats_chunk(0, 1)
            stats_chunk(0, 2)
            stats_chunk(0, 3)
            stage2_b(0)
            for q in range(4):
                qkv_isl(q)
                stats_chunk(1, q)
            stage2_b(1)
            for q in range(4):
                qkv_isl(4 + q)

            # ---------------- attention + late normalize/outproj ----------------
            o2tb = [persist.tile([128, N], bf16, tag="o2t", bufs=B, name=f"o2t{b}")
                    for b in range(B)]

            def attn_isl(b, isl):
                po_h0 = ps_po()
                po_h1 = ps_po()
                qsl = slice(isl * 512, (isl + 1) * 512)
                for jt in range(JT):
                    ksl = slice(jt * 128, (jt + 1) * 128)
                    st2 = ps_st2()
                    nc.tensor.matmul(st2[:, 0:512], k2Tb[b][0:64, ksl], q2Tb[b][0:64, qsl],
                                     start=True, stop=True)
                    nc.tensor.matmul(st2[:, 512:1024], k2Tb[b][64:128, ksl], q2Tb[b][64:128, qsl],
                                     start=True, stop=True)
                    pt2 = work.tile([128, 1024], bf16, tag="pt2", bufs=4)
                    nc.scalar.activation(pt2[:], st2[:], AF.Exp, scale=DH ** -0.5)
                    va, vh = V2[b * JT + jt]
                    fl = (jt == 0), (jt == JT - 1)
                    nc.tensor.matmul(po_h0[0:65, :], va[:], pt2[:, 0:512],
                                     start=fl[0], stop=fl[1])
                    nc.tensor.matmul(po_h1[0:65, :], vh[:], pt2[:, 512:1024],
                                     start=fl[0], stop=fl[1])
                return po_h0, po_h1

            def evac(po_h0, po_h1):
                # denominator rows + reciprocals first: they gate the
                # normalize/outproj chain, the bulk copies do not
                rin0 = work.tile([1, 512], f32, tag="rin", bufs=2)
                nc.vector.tensor_copy(rin0[:], po_h0[64:65, :])        # shifted
                rin1 = work.tile([1, 512], f32, tag="rin", bufs=2)
                nc.vector.tensor_copy(rin1[:], po_h1[64:65, :])        # shifted
                rp0 = work.tile([1, 512], f32, tag="rp", bufs=2)
                nc.vector.reciprocal_approx_fast(rp0[:], rin0[:])
                rp1 = work.tile([1, 512], f32, tag="rp", bufs=2)
                nc.vector.reciprocal_approx_fast(rp1[:], rin1[:])
                ob = work.tile([128, 512], bf16, tag="ob", bufs=2)
                nc.vector.tensor_copy(ob[0:64, :], po_h0[0:64, :])
                nc.vector.tensor_copy(ob[64:128, :], po_h1[0:64, :])   # shifted
                # moving rows for the r-broadcast matmul live at partitions 0
                # and 32 (32-aligned); rows 1..31 zeroed, killed by ones2v=0.
                rpb = work.tile([33, 512], bf16, tag="rpb", bufs=2)
                nc.vector.memset(rpb[0:32, :], 0.0)
                nc.vector.tensor_copy(rpb[0:1, :], rp0[:])
                nc.vector.tensor_copy(rpb[32:33, :], rp1[:])
                return ob, rpb

            def normout(b, isl, ob, rpb):
                osl = slice(isl * 512, (isl + 1) * 512)
                gsl = slice(b * N + isl * 512, b * N + (isl + 1) * 512)
                pr = ps_sm()
                nc.tensor.matmul(pr[:], ones2v[:], rpb[:], start=True, stop=True)
                o2t = o2tb[b]
                nc.vector.tensor_tensor(o2t[0:64, osl], ob[0:64, :], pr[0:64, :], op=ALU.mult)
                nc.vector.tensor_tensor(o2t[64:128, osl], ob[64:128, :], pr[64:128, :], op=ALU.mult)
                for ncx in range(8):
                    py = ps_sm()
                    nc.tensor.matmul(py[:], wo_bf[:, ncx * 128:(ncx + 1) * 128],
                                     o2t[:, osl], start=True, stop=True)
                    yb = work.tile([128, 512], bf16, tag="ysb", bufs=4)
                    nc.vector.tensor_copy(yb[:], py[:])
                    eng = nc.sync if ncx % 2 == 0 else nc.gpsimd
                    eng.dma_start(yT_out[ncx * 128:(ncx + 1) * 128, gsl], yb[:])

            pending = None
            for b in range(B):
                for isl in range(4):
                    po_h0, po_h1 = attn_isl(b, isl)
                    if pending is not None:
                        normout(*pending)
                    e = evac(po_h0, po_h1)
                    pending = (b, isl, e[0], e[1])
            normout(*pending)

    nc.compile()
    return nc


_NC_CACHE = None


def _get_nc():
    global _NC_CACHE
    if _NC_CACHE is None:
        _NC_CACHE = build_program()
    return _NC_CACHE


def make_in_maps(x, conditioning_embeddings, gamma, cond_W, cond_b, Wq, Wkv, Wo):
    x = np.asarray(x, np.float32)
    ce = np.asarray(conditioning_embeddings, np.float32)
    gamma = np.asarray(gamma, np.float32)
    cond_W = np.asarray(cond_W, np.float32)
    cond_b = np.asarray(cond_b, np.float32)
    Wq = np.asarray(Wq, np.float32)
    Wkv = np.asarray(Wkv, np.float32)
    Wo = np.asarray(Wo, np.float32)

    bf = ml_dtypes.bfloat16
    xT = np.ascontiguousarray(x.reshape(TOK, DIM).T).astype(bf)
    xN = np.ascontiguousarray(x.reshape(TOK, DIM)).astype(bf)
    ceT = np.ascontiguousarray(ce.reshape(B, KT, 128).transpose(2, 1, 0).reshape(128, 2 * KT))
    gammaT = np.ascontiguousarray(gamma.reshape(KT, 128).T)
    condb2 = np.ascontiguousarray(np.broadcast_to(cond_b, (2, 2 * DIM)))
    condW_bf = cond_W.astype(bf)
    in_maps = []
    for c in range(NCORES):
        cs = slice(128 * c, 128 * (c + 1))
        wqkv_c = np.ascontiguousarray(
            np.concatenate([Wq[:, cs], Wkv[:, cs], Wkv[:, 1024 + 128 * c:1024 + 128 * (c + 1)]], axis=1)
        ).astype(bf)
        in_maps.append({
            "xT": xT,
            "xN": xN,
            "ceT": ceT,
            "gammaT": gammaT,
            "condW": condW_bf,
            "condb": condb2,
            "wqkv": wqkv_c,
            "wo": np.ascontiguousarray(Wo[cs, :]).astype(bf),
        })
    return in_maps


def kernel(**inputs) -> np.ndarray:
    nc = _get_nc()
    in_maps = make_in_maps(**inputs)
    res = run_bass_kernel_spmd(nc, in_maps, core_ids=list(range(NCORES)))
    acc = np.zeros((DIM, TOK), np.float32)
    for core in res.results:
        acc += np.asarray(core["yT"]).astype(np.float32)
    return np.ascontiguousarray(acc.T).reshape(B, N, DIM)


# revision 36
# speedup vs baseline: 1.0264x; 1.0264x over previous
"""Trainium2 Bass kernel for nn_Attention_40037685133427.

FiLM-conditioned LayerNorm + 16-head self-attention (B=2, N=2048, D=1024),
tensor-parallel over 8 NeuronCores: core c owns heads {2c, 2c+1}.

v2 redesign, driven by microbenchmarks on this silicon (back-to-back 512-col
bf16 matmuls issue every ~216ns; 64-contraction quadrant pairs co-execute;
engine access patterns must start at 32-aligned partitions):
  - LN stats via DVE bn_stats/bn_aggr on a token-major copy of x (no PE
    matmuls, no serial chains, no DRAM round trips); batched Newton rsqrt.
  - per-token (u=rstd, m=mean*rstd, sd=1/u) row triplets are PE-transposed
    once per slice; u broadcasts to a [128,512] tile via a selector matmul,
    and (m, sd) ride the QKV accumulation as PE outer-products against a
    [3,128] (0, -sum(W_g), sum(beta'W)) stationary, so the whole LN+FiLM
    correction costs one DVE multiply per projection.
  - QKV runs on raw x as 3 bank-interleaved accumulation chains.
  - attention: S^T = K Q^T head-pairs co-execute in PE quadrants; one fused
    exp per (jt, slice) on ACT (scale folded, no max subtraction); attn@V
    uses 65-col stationaries [V | ones] so both heads' softmax denominators
    ride along; evacuation uses partition-shifted DVE copies, fast-approx
    reciprocals, and a zero-padded [33,128] selector matmul broadcasts 1/den.
  - normalize + output projection are emitted one slice late so their PE work
    never blocks the (ACT-exp-bound) attention stream; the steady-state
    attention phase runs with zero exp stalls (~142us for 128 exp tiles).
Host sums the 8 partial y^T outputs (row-split Wo => partial sums).
FiLM weights load as 8 wide DMAs on the sync queue feeding 4 parallel
accumulation chains; the scalar DMA queue is reserved for the token-major
x tiles so bn_stats starts immediately.
Measured: 320.1us HW exec (traced) per core (prep ~156us PE-bound, attention
~143us at the ACT-exp floor with zero stalls), rel L2 error 0.0054 vs fp32
reference; baseline v1 was 614us traced / 540.7us untraced.
"""

import sys

sys.path.insert(0, "/opt/trn_rl_repo")

import numpy as np
import ml_dtypes

import concourse.bass as bass
from concourse import bacc
import concourse.tile as tile
from concourse import mybir
from concourse.bass_utils import run_bass_kernel_spmd
from concourse.masks import make_identity

f32 = mybir.dt.float32
bf16 = mybir.dt.bfloat16
fp16 = mybir.dt.float16
AF = mybir.ActivationFunctionType
ALU = mybir.AluOpType

B, N, DIM = 2, 2048, 1024
HEADS, DH = 16, 64
TOK = B * N            # 4096 tokens, batch-major
KT = DIM // 128        # 8 k-tiles over the model dim
NSL = TOK // 512       # 8 token slices of 512
JT = N // 128          # 16 key tiles per batch
COND = 1024
NCORES = 8


def build_program():
    nc = bacc.Bacc("TRN2", target_bir_lowering=False, debug=False)

    xT = nc.dram_tensor("xT", [DIM, TOK], bf16, kind="ExternalInput").ap()
    xN = nc.dram_tensor("xN", [TOK, DIM], bf16, kind="ExternalInput").ap()
    ceT = nc.dram_tensor("ceT", [128, 2 * KT], f32, kind="ExternalInput").ap()
    gammaT = nc.dram_tensor("gammaT", [128, KT], f32, kind="ExternalInput").ap()
    condW = nc.dram_tensor("condW", [COND, 2 * DIM], bf16, kind="ExternalInput").ap()
    condb = nc.dram_tensor("condb", [2, 2 * DIM], f32, kind="ExternalInput").ap()
    wqkv = nc.dram_tensor("wqkv", [DIM, 384], bf16, kind="ExternalInput").ap()
    wo = nc.dram_tensor("wo", [128, DIM], bf16, kind="ExternalInput").ap()

    yT_out = nc.dram_tensor("yT", [DIM, TOK], bf16, kind="ExternalOutput").ap()

    with tile.TileContext(nc) as tc:
        with (
            tc.tile_pool(name="const", bufs=1) as const,
            tc.tile_pool(name="persist", bufs=1) as persist,
            tc.tile_pool(name="big", bufs=1) as bigp,
            tc.tile_pool(name="work", bufs=3) as work,
            tc.tile_pool(name="ps", bufs=8, space="PSUM") as ps,
        ):
            # PSUM tags: st2 2x[128,1024] (4 banks), po 2x[128,512] (2 banks),
            # ps 2x[128,512] (2 banks) = 8 banks
            def ps_st2(shape=(128, 1024), dtype=f32):
                return ps.tile(list(shape), dtype, tag="st2", bufs=2, name="st2t")

            def ps_po(shape=(128, 512), dtype=f32):
                return ps.tile(list(shape), dtype, tag="po", bufs=2, name="pot")

            def ps_sm(shape=(128, 512), dtype=f32):
                return ps.tile(list(shape), dtype, tag="ps", bufs=2, name="pst")

            # ---------------- constants / weights ----------------
            ident = const.tile([128, 128], bf16)
            make_identity(nc, ident[:])
            ident32 = const.tile([128, 128], f32)
            make_identity(nc, ident32[:])
            ones_col = const.tile([128, 1], bf16)
            nc.vector.memset(ones_col[:], 1.0)
            ones_col_h = const.tile([128, 1], fp16)
            nc.vector.memset(ones_col_h[:], 1.0)
            ones1 = const.tile([1, 128], bf16)
            nc.vector.memset(ones1[:], 1.0)
            # [33,128] selector: row 0 -> partitions 0:64 (head0), row 32 ->
            # 64:128 (head1); rows 1..31 are zero so junk moving rows vanish.
            ones2v = const.tile([33, 128], bf16)
            nc.vector.memset(ones2v[:], 0.0)
            nc.vector.memset(ones2v[0:1, 0:64], 1.0)
            nc.vector.memset(ones2v[32:33, 64:128], 1.0)

            wo_bf = persist.tile([128, DIM], bf16, tag="wo")
            nc.sync.dma_start(wo_bf[:], wo)
            wg_raw = []
            for kt in range(KT):
                wg = persist.tile([128, 384], bf16, tag="wg", bufs=KT, name=f"wg{kt}")
                nc.sync.dma_start(wg[:], wqkv[kt * 128:(kt + 1) * 128, :])
                wg_raw.append(wg)
            # x loaded as 16 [128, 2048] tiles: (kt, batch-half)
            x16 = [[None, None] for _ in range(KT)]
            for kt in range(KT):
                for bh in range(2):
                    xr = bigp.tile([128, 2048], bf16, tag="x16", bufs=16,
                                   name=f"x{kt}_{bh}")
                    nc.sync.dma_start(xr[:], xT[kt * 128:(kt + 1) * 128,
                                            bh * 2048:(bh + 1) * 2048])
                    x16[kt][bh] = xr

            def xsl(kt, isl):
                bh, q = isl // 4, isl % 4
                return x16[kt][bh][:, q * 512:(q + 1) * 512]

            gam = const.tile([128, KT], f32)
            nc.gpsimd.dma_start(gam[:], gammaT)
            cet = const.tile([128, 2 * KT], f32)
            nc.gpsimd.dma_start(cet[:], ceT)

            # ---------------- FiLM conditioning ----------------
            sil = const.tile([128, 2 * KT], f32)
            # silu(x) = x / (1 + exp(-x)) via Exp (single ACT table set)
            nc.scalar.activation(sil[:], cet[:], AF.Exp, scale=-1.0)
            nc.vector.tensor_scalar(sil[:], sil[:], 1.0, None, ALU.add)
            nc.vector.reciprocal(sil[:], sil[:])
            nc.vector.tensor_tensor(sil[:], sil[:], cet[:], op=ALU.mult)
            sil_bf = const.tile([128, 2 * KT], bf16)
            nc.vector.tensor_copy(sil_bf[:], sil[:])

            gp = const.tile([128, 2 * KT], f32)   # scale^T, col = b*KT + kt
            bp = const.tile([128, 2 * KT], f32)   # shift^T
            # 4 film chains accumulate in parallel over kt so one wide condW
            # DMA per kt (on the otherwise-idle sync queue) feeds all of them
            pcs = [ps_po((2, 512)), ps_po((2, 512)), ps_sm((2, 512)), ps_sm((2, 512))]
            for kt in range(KT):
                cwk = work.tile([128, 2048], bf16, tag="cwk", bufs=2)
                nc.sync.dma_start(cwk[:], condW[kt * 128:(kt + 1) * 128, :])
                for cs in range(4):
                    nc.tensor.matmul(pcs[cs][:], sil_bf[:, 2 * kt:2 * kt + 2],
                                     cwk[:, cs * 512:(cs + 1) * 512],
                                     start=(kt == 0), stop=(kt == KT - 1))
            csls = []
            for cs in range(4):
                cbw = work.tile([2, 512], f32, tag="cbw", bufs=4)
                nc.gpsimd.dma_start(cbw[:], condb[:, cs * 512:(cs + 1) * 512])
                csl = work.tile([2, 512], f32, tag="csl", bufs=4)
                nc.vector.tensor_tensor(csl[:], pcs[cs][:], cbw[:], op=ALU.add)
                csls.append(csl)
            for cs in range(4):
                dst = gp if cs < 2 else bp
                for c in range(4):
                    tp = ps_sm((128, 2))
                    nc.tensor.matmul(tp[:], csls[cs][0:2, c * 128:(c + 1) * 128],
                                     ident32[0:2, 0:2], is_transpose=True,
                                     start=True, stop=True)
                    k = (4 * cs + c) % KT
                    nc.vector.tensor_copy(dst[:, k::KT], tp[:])
            gpf = const.tile([128, 2 * KT], f32)
            nc.vector.tensor_scalar(gpf[:], gp[:], 1.0, None, ALU.add)
            for b in range(B):
                sl = slice(b * KT, (b + 1) * KT)
                nc.vector.tensor_tensor(gpf[:, sl], gpf[:, sl], gam[:], op=ALU.mult)
            bpb = const.tile([128, 2 * KT], bf16)
            nc.vector.tensor_copy(bpb[:], bp[:])

            # ---------------- per-batch folded QKV weights + column sums ----------------
            wscaled = [[None] * KT for _ in range(B)]
            wgs_neg, wbs = [], []
            for b in range(B):
                pgs = ps_po((1, 512))
                pbs = ps_po((1, 512))
                for kt in range(KT):
                    col = b * KT + kt
                    nc.tensor.matmul(pbs[0:1, 0:384], bpb[:, col:col + 1], wg_raw[kt][:],
                                     start=(kt == 0), stop=(kt == KT - 1))
                    wsb = persist.tile([128, 384], bf16, tag="wsb", bufs=B * KT,
                                       name=f"wsb{b}_{kt}")
                    nc.vector.tensor_scalar(wsb[:], wg_raw[kt][:], gpf[:, col:col + 1],
                                            None, ALU.mult)
                    nc.tensor.matmul(pgs[0:1, 0:384], ones_col[:], wsb[:],
                                     start=(kt == 0), stop=(kt == KT - 1))
                    wscaled[b][kt] = wsb
                gsr = work.tile([1, 512], f32, tag="wrow", bufs=2)
                nc.vector.tensor_copy(gsr[0:1, 0:384], pgs[0:1, 0:384])
                bsr = work.tile([1, 512], f32, tag="wrow", bufs=2)
                nc.vector.tensor_copy(bsr[0:1, 0:384], pbs[0:1, 0:384])
                wtp = ps_sm((128, 3))
                for c in range(3):
                    nc.tensor.matmul(wtp[:, c:c + 1], gsr[0:1, c * 128:(c + 1) * 128],
                                     ident32[0:1, 0:1], is_transpose=True,
                                     start=True, stop=True)
                wg_n = const.tile([128, 3], f32, name=f"wgn{b}")
                nc.vector.tensor_scalar(wg_n[:], wtp[:], -1.0, None, ALU.mult)
                wgs_neg.append(wg_n)
                wtp2 = ps_sm((128, 3))
                for c in range(3):
                    nc.tensor.matmul(wtp2[:, c:c + 1], bsr[0:1, c * 128:(c + 1) * 128],
                                     ident32[0:1, 0:1], is_transpose=True,
                                     start=True, stop=True)
                wb_c = const.tile([128, 3], f32, name=f"wbc{b}")
                nc.vector.tensor_copy(wb_c[:], wtp2[:])
                wbs.append(wb_c)

            # [3,128] per-(batch,proj) correction stationaries: rows
            # (0, wgs_neg, wbs); contracted with the (u, m, 1) row triplet
            # they add m*wgs_neg + wbs into the QKV accumulation on the PE.
            wqksel = [[None] * 3 for _ in range(B)]
            for b in range(B):
                for p in range(3):
                    wqc = work.tile([128, 3], bf16, tag="wqc", bufs=2)
                    nc.vector.memset(wqc[:, 0:1], 0.0)
                    nc.vector.tensor_copy(wqc[:, 1:2], wgs_neg[b][:, p:p + 1])
                    nc.vector.tensor_copy(wqc[:, 2:3], wbs[b][:, p:p + 1])
                    pwq = ps_sm((3, 128), bf16)
                    nc.tensor.matmul(pwq[:], wqc[:], ident[:],
                                     is_transpose=True, start=True, stop=True)
                    wsel = const.tile([3, 128], bf16, name=f"wqksel{b}_{p}")
                    nc.vector.tensor_copy(wsel[:], pwq[:])
                    wqksel[b][p] = wsel

            # ---------------- LN stats via DVE bn_stats on token-major x ----------------
            # bn_stats/bn_aggr produce per-token (mean, var) directly; no PE
            # matmuls, no serial accumulation chains, no transposes needed.
            stagMV = [const.tile([128, 32], f32, name=f"stagmv{b}") for b in range(B)]

            def stats_chunk(b, q4):
                for q in range(4 * q4, 4 * q4 + 4):   # 4 token-tiles of 128
                    tt = b * 16 + q
                    xn = work.tile([128, DIM], bf16, tag="xn", bufs=6)
                    nc.scalar.dma_start(xn[:], xN[tt * 128:(tt + 1) * 128, :])
                    bnb = work.tile([128, 12], f32, tag="bnb", bufs=3)
                    nc.vector.bn_stats(bnb[:, 0:6], xn[:, 0:512])
                    nc.vector.bn_stats(bnb[:, 6:12], xn[:, 512:1024])
                    nc.vector.bn_aggr(stagMV[b][:, 2 * q:2 * q + 2], bnb[:])

            # ---------------- stage 2: batched Newton rsqrt + broadcast back ----------------
            # per-slice (u, m, 1) row triplets: u broadcasts to U_sb via a
            # selector matmul; (m, 1) rows ride the QKV accumulation as
            # outer-product corrections (see wqksel below).
            U_sb, umr_sb = [None] * NSL, [None] * NSL
            selc = const.tile([128, 3], bf16)
            nc.vector.memset(selc[:, 0:1], 1.0)
            nc.vector.memset(selc[:, 1:3], 0.0)
            sel_u = const.tile([3, 128], bf16)
            psel = ps_sm((3, 128), bf16)
            nc.tensor.matmul(psel[:], selc[:], ident[:],
                             is_transpose=True, start=True, stop=True)
            nc.vector.tensor_copy(sel_u[:], psel[:])

            def stage2_b(b):
                mean = work.tile([128, 16], f32, tag="mean", bufs=2)
                nc.vector.tensor_copy(mean[:], stagMV[b][:, 0::2])
                var = work.tile([128, 16], f32, tag="var", bufs=2)
                nc.vector.tensor_scalar(var[:], stagMV[b][:, 1::2], 1.0, 1e-5, ALU.mult, ALU.add)
                y = work.tile([128, 16], f32, tag="yt", bufs=2)
                nc.vector.tensor_scalar(y[:], var[:], -0.5, 1.5, ALU.mult, ALU.add)
                t = work.tile([128, 16], f32, tag="tt", bufs=2)
                for _ in range(2):
                    nc.vector.tensor_tensor(t[:], y[:], y[:], op=ALU.mult)
                    nc.vector.tensor_tensor(t[:], t[:], var[:], op=ALU.mult)
                    nc.vector.tensor_scalar(t[:], t[:], -0.5, 1.5, ALU.mult, ALU.add)
                    nc.vector.tensor_tensor(y[:], y[:], t[:], op=ALU.mult)
                m = work.tile([128, 16], f32, tag="mt", bufs=2)
                nc.vector.tensor_tensor(m[:], mean[:], y[:], op=ALU.mult)
                sd = work.tile([128, 16], f32, tag="sd", bufs=2)
                nc.vector.tensor_tensor(sd[:], var[:], y[:], op=ALU.mult)
                stg2 = const.tile([128, 48], bf16, name=f"stg2_{b}")
                nc.vector.tensor_copy(stg2[:, 0::3], y[:])
                nc.vector.tensor_copy(stg2[:, 1::3], m[:])
                nc.vector.tensor_copy(stg2[:, 2::3], sd[:])
                for q in range(4):
                    isl = b * 4 + q
                    umps = ps_sm((3, 512), bf16)
                    for c in range(4):
                        tt = q * 4 + c
                        nc.tensor.matmul(umps[0:3, c * 128:(c + 1) * 128],
                                         stg2[:, 3 * tt:3 * tt + 3], ident[:],
                                         is_transpose=True, start=True, stop=True)
                    umr = persist.tile([3, 512], bf16, tag="umr", bufs=NSL,
                                       name=f"umr{isl}")
                    nc.vector.tensor_copy(umr[:], umps[:])
                    umr_sb[isl] = umr
                    pu = ps_sm()
                    nc.tensor.matmul(pu[:], sel_u[:], umr[:], start=True, stop=True)
                    ub = persist.tile([128, 512], bf16, tag="Usb", bufs=NSL)
                    nc.vector.tensor_copy(ub[:], pu[:])
                    U_sb[isl] = ub

            # ---------------- QKV on raw x (LN+FiLM affine folded into weights) ----------------
            # per-batch tensors so batch-1 writes never alias batch-0 reads
            q2Tb = [persist.tile([128, N], bf16, tag="q2T", bufs=B, name=f"q2T{b}")
                    for b in range(B)]
            k2Tb = [persist.tile([128, N], bf16, tag="k2T", bufs=B, name=f"k2T{b}")
                    for b in range(B)]
            V2 = [None] * (B * JT)

            def qkv_isl(isl):
                b, q = isl // 4, isl % 4
                sl = slice(q * 512, (q + 1) * 512)
                # interleaved accumulation chains on 3 PSUM banks; st2 slots
                # are free before attention, widening the rotation so chain
                # k+1 never waits on chain k's evacuation
                pqs = {2: ps_st2((128, 512)), 1: ps_st2((128, 512)), 0: ps_po()}
                for kt in range(KT):
                    for p in (2, 1, 0):
                        nc.tensor.matmul(pqs[p][:], wscaled[b][kt][:, p * 128:(p + 1) * 128],
                                         xsl(kt, isl),
                                         start=(kt == 0), stop=False)
                for p in (2, 1, 0):
                    nc.tensor.matmul(pqs[p][:], wqksel[b][p][:], umr_sb[isl][:],
                                     start=False, stop=True)
                # evacuate all three chains first so the PSUM slots free up
                vtile = work.tile([128, 512], bf16, tag="vtile", bufs=3)
                nc.vector.tensor_tensor(vtile[:], pqs[2][:], U_sb[isl][:], op=ALU.mult)
                nc.vector.tensor_tensor(k2Tb[b][:, sl], pqs[1][:], U_sb[isl][:], op=ALU.mult)
                nc.vector.tensor_tensor(q2Tb[b][:, sl], pqs[0][:], U_sb[isl][:], op=ALU.mult)
                for q4 in range(4):
                    jt = isl * 4 + q4
                    pv = ps_sm((128, 128), bf16)
                    nc.tensor.matmul(pv[:], vtile[:, q4 * 128:(q4 + 1) * 128],
                                     ident[:], is_transpose=True,
                                     start=True, stop=True)
                    va = persist.tile([128, 65], bf16, tag="Va0", bufs=B * JT)
                    nc.vector.tensor_copy(va[:, 0:64], pv[:, 0:64])
                    nc.vector.memset(va[:, 64:65], 1.0)
                    vh = persist.tile([128, 65], bf16, tag="Vh1", bufs=B * JT)
                    nc.vector.tensor_copy(vh[:, 0:64], pv[:, 64:128])
                    nc.vector.memset(vh[:, 64:65], 1.0)
                    V2[jt] = (va, vh)

            # prep: batch-0 chain, then batch-1; bn of b1 overlaps b0 QKV
            stats_chunk(0, 0)
            stats_chunk(0, 1)
            stats_chunk(0, 2)
            stats_chunk(0, 3)
            stage2_b(0)
            for q in range(4):
                qkv_isl(q)
                stats_chunk(1, q)
            stage2_b(1)
            for q in range(4):
                qkv_isl(4 + q)

            # ---------------- attention + late normalize/outproj ----------------
            o2tb = [persist.tile([128, N], bf16, tag="o2t", bufs=B, name=f"o2t{b}")
                    for b in range(B)]

            def attn_isl(b, isl):
                po_h0 = ps_po()
                po_h1 = ps_po()
                qsl = slice(isl * 512, (isl + 1) * 512)
                for jt in range(JT):
                    ksl = slice(jt * 128, (jt + 1) * 128)
                    st2 = ps_st2()
                    nc.tensor.matmul(st2[:, 0:512], k2Tb[b][0:64, ksl], q2Tb[b][0:64, qsl],
                                     start=True, stop=True)
                    nc.tensor.matmul(st2[:, 512:1024], k2Tb[b][64:128, ksl], q2Tb[b][64:128, qsl],
                                     start=True, stop=True)
                    pt2 = work.tile([128, 1024], bf16, tag="pt2", bufs=4)
                    nc.scalar.activation(pt2[:], st2[:], AF.Exp, scale=DH ** -0.5)
                    va, vh = V2[b * JT + jt]
                    fl = (jt == 0), (jt == JT - 1)
                    nc.tensor.matmul(po_h0[0:65, :], va[:], pt2[:, 0:512],
                                     start=fl[0], stop=fl[1])
                    nc.tensor.matmul(po_h1[0:65, :], vh[:], pt2[:, 512:1024],
                                     start=fl[0], stop=fl[1])
                return po_h0, po_h1

            def evac(po_h0, po_h1):
                ob = work.tile([128, 512], bf16, tag="ob", bufs=2)
                nc.vector.tensor_copy(ob[0:64, :], po_h0[0:64, :])
                nc.vector.tensor_copy(ob[64:128, :], po_h1[0:64, :])   # shifted
                rin0 = work.tile([1, 512], f32, tag="rin", bufs=2)
                nc.vector.tensor_copy(rin0[:], po_h0[64:65, :])        # shifted
                rin1 = work.tile([1, 512], f32, tag="rin", bufs=2)
                nc.vector.tensor_copy(rin1[:], po_h1[64:65, :])        # shifted
                rp0 = work.tile([1, 512], f32, tag="rp", bufs=2)
                nc.vector.reciprocal_approx_fast(rp0[:], rin0[:])
                rp1 = work.tile([1, 512], f32, tag="rp", bufs=2)
                nc.vector.reciprocal_approx_fast(rp1[:], rin1[:])
                # moving rows for the r-broadcast matmul live at partitions 0
                # and 32 (32-aligned); rows 1..31 zeroed, killed by ones2v=0.
                rpb = work.tile([33, 512], bf16, tag="rpb", bufs=2)
                nc.vector.memset(rpb[0:32, :], 0.0)
                nc.vector.tensor_copy(rpb[0:1, :], rp0[:])
                nc.vector.tensor_copy(rpb[32:33, :], rp1[:])
                return ob, rpb

            def normout(b, isl, ob, rpb):
                osl = slice(isl * 512, (isl + 1) * 512)
                gsl = slice(b * N + isl * 512, b * N + (isl + 1) * 512)
                pr = ps_sm()
                nc.tensor.matmul(pr[:], ones2v[:], rpb[:], start=True, stop=True)
                o2t = o2tb[b]
                nc.vector.tensor_tensor(o2t[0:64, osl], ob[0:64, :], pr[0:64, :], op=ALU.mult)
                nc.vector.tensor_tensor(o2t[64:128, osl], ob[64:128, :], pr[64:128, :], op=ALU.mult)
                for ncx in range(8):
                    py = ps_sm()
                    nc.tensor.matmul(py[:], wo_bf[:, ncx * 128:(ncx + 1) * 128],
                                     o2t[:, osl], start=True, stop=True)
                    yb = work.tile([128, 512], bf16, tag="ysb", bufs=3)
                    nc.vector.tensor_copy(yb[:], py[:])
                    nc.sync.dma_start(yT_out[ncx * 128:(ncx + 1) * 128, gsl], yb[:])

            pending = None
            for b in range(B):
                for isl in range(4):
                    po_h0, po_h1 = attn_isl(b, isl)
                    if pending is not None:
                        normout(*pending)
                    e = evac(po_h0, po_h1)
                    pending = (b, isl, e[0], e[1])
            normout(*pending)

    nc.compile()
    return nc


_NC_CACHE = None


def _get_nc():
    global _NC_CACHE
    if _NC_CACHE is None:
        _NC_CACHE = build_program()
    return _NC_CACHE


def make_in_maps(x, conditioning_embeddings, gamma, cond_W, cond_b, Wq, Wkv, Wo):
    x = np.asarray(x, np.float32)
    ce = np.asarray(conditioning_embeddings, np.float32)
    gamma = np.asarray(gamma, np.float32)
    cond_W = np.asarray(cond_W, np.float32)
    cond_b = np.asarray(cond_b, np.float32)
    Wq = np.asarray(Wq, np.float32)
    Wkv = np.asarray(Wkv, np.float32)
    Wo = np.asarray(Wo, np.float32)

    bf = ml_dtypes.bfloat16
    xT = np.ascontiguousarray(x.reshape(TOK, DIM).T).astype(bf)
    xN = np.ascontiguousarray(x.reshape(TOK, DIM)).astype(bf)
    ceT = np.ascontiguousarray(ce.reshape(B, KT, 128).transpose(2, 1, 0).reshape(128, 2 * KT))
    gammaT = np.ascontiguousarray(gamma.reshape(KT, 128).T)
    condb2 = np.ascontiguousarray(np.broadcast_to(cond_b, (2, 2 * DIM)))
    condW_bf = cond_W.astype(bf)
    in_maps = []
    for c in range(NCORES):
        cs = slice(128 * c, 128 * (c + 1))
        wqkv_c = np.ascontiguousarray(
            np.concatenate([Wq[:, cs], Wkv[:, cs], Wkv[:, 1024 + 128 * c:1024 + 128 * (c + 1)]], axis=1)
        ).astype(bf)
        in_maps.append({
            "xT": xT,
            "xN": xN,
            "ceT": ceT,
            "gammaT": gammaT,
            "condW": condW_bf,
            "condb": condb2,
            "wqkv": wqkv_c,
            "wo": np.ascontiguousarray(Wo[cs, :]).astype(bf),
        })
    return in_maps


def kernel(**inputs) -> np.ndarray:
    nc = _get_nc()
    in_maps = make_in_maps(**inputs)
    res = run_bass_kernel_spmd(nc, in_maps, core_ids=list(range(NCORES)))
    acc = np.zeros((DIM, TOK), np.float32)
    for core in res.results:
        acc += np.asarray(core["yT"]).astype(np.float32)
    return np.ascontiguousarray(acc.T).reshape(B, N, DIM)
